# revision 1
# baseline (speedup 1.0000x reference)
"""Trainium2 Bass kernel for additive (Bahdanau) attention.

  context[b] = sum_t softmax_t( v . tanh(We @ enc[b,t] + Wd @ dec[b] + bias) ) * enc[b,t]

Shapes (hardcoded): enc_out [64, 2048, 1024] f32, dec_state [64, 1024] f32,
W_weight [1024, 2048], W_bias [1024], v_weight [1, 1024].  Output [64, 1024].

Sharding: data-parallel over batch across 8 NeuronCores (8 batches/core).
Host prep: We^T relayout, v replication, and the tiny bias term
z = Wd @ dec + W_bias (0.05% of FLOPs) replicated to 128 partitions.

Per-core, one global software pipeline over 128 row-tiles [128t x 1024e].
All matmuls use float32r (TF32-class, ~11 mantissa bits, 1 cycle/row on the
PE vs 4 for plain fp32; fp32 accumulate in PSUM).  PE stream per step k:
  transpose(k)  8x 128x128 is_transpose matmuls of the enc tile -> PSUM
  ctx(k-2)      2x N=512 matmuls: ctx_unnorm += exp(s)^T @ X
  proj(k-1)     16x N=512 matmuls: proj = X @ We^T (K over 8 e-tiles)
so the PSUM->SBUF transpose copies (split ACT/DVE) and the DVE/ACT epilogue
(z-add, tanh, fused v-mult+reduce via scalar_tensor_tensor, exp) of one step
overlap the next step's PE work.  Softmax needs no max-subtraction
(|scores| <= sum|v| <= 32, exp safe in fp32), so exp weights are final and
ctx_unnorm accumulates across all 16 t-tiles in PSUM; one reciprocal scale
per batch normalizes.  Measured: ~675 us/core, rel err ~1.5e-4 (f32r rounding).
"""

import os
import sys

sys.path.insert(0, "/opt/trn_rl_repo")

from contextlib import ExitStack

import numpy as np

import concourse.bass as bass
import concourse.tile as tile
from concourse import bacc, mybir
from concourse.bass import ts
from concourse.bass_utils import run_bass_kernel_spmd

F32 = mybir.dt.float32
F32R = mybir.dt.float32r

B, T, E, D = 64, 2048, 1024, 1024
CORES = 8
BL = B // CORES           # batches per core
P = 128                   # partitions
TT = T // P               # t-tiles per batch (16)
ET = E // P               # e-tiles (K tiles) per row-tile (8)
CTX_LAG = 2               # t-tiles of lag before emitting ctx matmuls


def _build_kernel(bl=BL, t_tiles=TT):
    nc = bacc.Bacc(
        "TRN2",
        target_bir_lowering=False,
        debug=False,
        num_devices=CORES,
    )
    t_rows = t_tiles * P

    enc = nc.declare_dram_parameter("enc", [bl, t_rows, E], F32R, isOutput=False)
    # We^T prearranged to [128, ET*1024]: block j holds We.T[j*128:(j+1)*128, :]
    wet = nc.declare_dram_parameter("wet", [P, ET * D], F32R, isOutput=False)
    # z = Wd @ dec[b] + bias, computed host-side, replicated across 128 partitions
    zrepp = nc.declare_dram_parameter("zrepp", [P, bl, D], F32, isOutput=False)
    vrep = nc.declare_dram_parameter("vrep", [P, D], F32, isOutput=False)
    ident = nc.declare_dram_parameter("ident", [P, P], F32R, isOutput=False)
    onesc = nc.declare_dram_parameter("onesc", [P, 1], F32, isOutput=False)
    out = nc.declare_dram_parameter("ctx_out", [bl, E], F32, isOutput=True)

    with tile.TileContext(nc) as tc, ExitStack() as ctx:
        const = ctx.enter_context(tc.tile_pool(name="const", bufs=1))
        xpool = ctx.enter_context(tc.tile_pool(name="x", bufs=4 + CTX_LAG))
        xtpool = ctx.enter_context(tc.tile_pool(name="xt", bufs=3))
        epool = ctx.enter_context(tc.tile_pool(name="e", bufs=3))
        small = ctx.enter_context(tc.tile_pool(name="small", bufs=2))

        ps_xt = ctx.enter_context(tc.tile_pool(name="ps_xt", bufs=2, space="PSUM"))
        ps_proj = ctx.enter_context(tc.tile_pool(name="ps_proj", bufs=2, space="PSUM"))
        ps_ctx = ctx.enter_context(tc.tile_pool(name="ps_ctx", bufs=2, space="PSUM"))

        # ---- resident constants. Order matters: the first enc tiles, identity and
        # zrep row 0 must not queue behind the 4MB of We^T on the sync queue.
        ident_sb = const.tile([P, P], F32R)
        nc.sync.dma_start(ident_sb[:], ident[:])
        n_pre = min(4, t_tiles)
        x_pre = [
            xpool.tile([P, E], F32R, tag="x", name=f"x_pre{i}") for i in range(n_pre)
        ]
        for i in range(n_pre):
            nc.sync.dma_start(x_pre[i][:], enc[0, ts(i, P), :])
        zrep_sb = const.tile([P, bl, D], F32)
        nc.sync.dma_start(zrep_sb[:, 0, :], zrepp[:, 0, :])
        vrep_sb = const.tile([P, D], F32)
        nc.sync.dma_start(vrep_sb[:], vrep[:])
        # weights as per-block tiles so consumers wait per 512KB block, not 4MB
        wet_t = []
        for j in range(ET):
            wj = const.tile([P, D], F32R, name=f"wet{j}")
            nc.sync.dma_start(wj[:], wet[:, j * D : (j + 1) * D])
            wet_t.append(wj)
        onesc_sb = const.tile([P, 1], F32)
        nc.sync.dma_start(onesc_sb[:], onesc[:])

        # ---- main loop: one global software pipeline over all (batch, t-tile) --
        total = bl * t_tiles
        state = {}

        def get_state(b):
            if b not in state:
                state[b] = dict(
                    s_all=small.tile([P, t_tiles], F32, tag="s", name=f"s_all_{b}"),
                    p_all=small.tile([P, t_tiles], F32, tag="p", name=f"p_all_{b}"),
                    ctx0=ps_ctx.tile([1, 512], F32, tag="ps_ctx", name=f"ctx0_{b}"),
                    ctx1=ps_ctx.tile([1, 512], F32, tag="ps_ctx", name=f"ctx1_{b}"),
                    x_tiles=[None] * t_tiles,
                    xt_sbs=[None] * t_tiles,
                    p_r_cols=[None] * t_tiles,
                )
            return state[b]

        def emit_load_transpose(b, i):
            st = get_state(b)
            if b == 0 and i < n_pre:
                x_tile = x_pre[i]
            else:
                x_tile = xpool.tile([P, E], F32R, tag="x")
                nc.sync.dma_start(x_tile[:], enc[b, ts(i, P), :])
            st["x_tiles"][i] = x_tile
            # transpose X tile 128x128-blockwise:
            # xt[e_loc, j*128 + t] = x[t, j*128+e_loc]; copies split ACT/DVE
            xt_sb = xtpool.tile([P, E], F32R, tag="xt_sb")
            st["xt_sbs"][i] = xt_sb
            for g in range(2):
                xt_ps = ps_xt.tile([P, 512], F32R, tag="ps_xt")
                for j4 in range(4):
                    j = g * 4 + j4
                    nc.tensor.transpose(
                        xt_ps[:, j4 * P : (j4 + 1) * P],
                        x_tile[:, j * P : (j + 1) * P],
                        ident_sb[:],
                    )
                if g == 0:
                    nc.scalar.copy(xt_sb[:, g * 512 : (g + 1) * 512], xt_ps[:])
                else:
                    nc.vector.tensor_copy(xt_sb[:, g * 512 : (g + 1) * 512], xt_ps[:])

        def emit_proj_epilogue(b, i):
            # proj[t, d] = sum_e x[t, e] * WeT[e, d]   (K over 8 e-tiles)
            st = get_state(b)
            xt_sb = st["xt_sbs"][i]
            proj_ps = ps_proj.tile([P, D], F32, tag="ps_proj")
            for j in range(ET):
                lhs = xt_sb[:, j * P : (j + 1) * P]
                nc.tensor.matmul(
                    proj_ps[:, 0:512], lhs, wet_t[j][:, 0:512],
                    start=(j == 0), stop=(j == ET - 1),
                )
                nc.tensor.matmul(
                    proj_ps[:, 512:D], lhs, wet_t[j][:, 512:D],
                    start=(j == 0), stop=(j == ET - 1),
                )
            # energy = tanh(proj + z); s = sum_d energy * v
            e_sb = epool.tile([P, D], F32, tag="e")
            nc.vector.tensor_add(e_sb[:], proj_ps[:], zrep_sb[:, b, :])
            nc.scalar.activation(e_sb[:], e_sb[:], mybir.ActivationFunctionType.Tanh)
            nc.vector.scalar_tensor_tensor(
                out=e_sb[:],
                in0=e_sb[:],
                scalar=1.0,
                in1=vrep_sb[:],
                op0=mybir.AluOpType.mult,
                op1=mybir.AluOpType.mult,
                accum_out=st["s_all"][:, i : i + 1],
            )
            nc.scalar.activation(
                st["p_all"][:, i : i + 1],
                st["s_all"][:, i : i + 1],
                mybir.ActivationFunctionType.Exp,
            )
            p_r = small.tile([P, 1], F32R, tag="pr")
            st["p_r_cols"][i] = p_r
            nc.vector.tensor_copy(p_r[:], st["p_all"][:, i : i + 1])

        def emit_ctx(b, i):
            # ctx_unnorm += p^T @ X  (contraction over the 128 t-rows)
            st = get_state(b)
            p_col = st["p_r_cols"][i][:]
            nc.tensor.matmul(
                st["ctx0"][:], p_col, st["x_tiles"][i][:, 0:512],
                start=(i == 0), stop=(i == t_tiles - 1),
            )
            nc.tensor.matmul(
                st["ctx1"][:], p_col, st["x_tiles"][i][:, 512:E],
                start=(i == 0), stop=(i == t_tiles - 1),
            )
            if i == t_tiles - 1:
                emit_batch_end(b)

        def emit_batch_end(b):
            # l = sum_t exp(s_t); ctx = ctx_unnorm / l
            st = state.pop(b)
            l_part = small.tile([P, 1], F32, tag="lp")
            nc.vector.tensor_reduce(
                l_part[:], st["p_all"][:],
                axis=mybir.AxisListType.X, op=mybir.AluOpType.add,
            )
            l_ps = ps_xt.tile([1, 1], F32, tag="ps_xt")
            nc.tensor.matmul(l_ps[:], l_part[:], onesc_sb[:])
            linv = small.tile([1, 1], F32, tag="linv")
            nc.vector.reciprocal(linv[:], l_ps[:])
            ctx_row = small.tile([1, E], F32, tag="ctxrow")
            nc.scalar.activation(
                ctx_row[:, 0:512], st["ctx0"][:],
                mybir.ActivationFunctionType.Copy, scale=linv[:],
            )
            nc.scalar.activation(
                ctx_row[:, 512:E], st["ctx1"][:],
                mybir.ActivationFunctionType.Copy, scale=linv[:],
            )
            nc.sync.dma_start(out[b : b + 1, :], ctx_row[:])

        # PE stream per step k: transp(k) -> ctx(k-2) -> proj(k-1); the psum->sbuf
        # transpose copies of step k overlap with proj(k-1) on ACT/DVE.
        for k in range(total + 2):
            if 0 < k <= bl - 1:
                nc.sync.dma_start(zrep_sb[:, k, :], zrepp[:, k, :])
            if k < total:
                emit_load_transpose(*divmod(k, t_tiles))
            if k - CTX_LAG >= 0:
                emit_ctx(*divmod(k - CTX_LAG, t_tiles))
            if k - 1 >= 0 and k - 1 < total:
                emit_proj_epilogue(*divmod(k - 1, t_tiles))

    nc.compile()
    return nc


def _prep_inputs(enc_out, dec_state, W_weight, W_bias, v_weight, bl=BL):
    """Host-side layout prep (transposes/replication + the tiny Wd@dec bias
    term, 0.05% of FLOPs) + per-core slicing."""
    enc_out = np.ascontiguousarray(enc_out, dtype=np.float32)
    dec_state = np.ascontiguousarray(dec_state, dtype=np.float32)
    W = np.asarray(W_weight, dtype=np.float32)
    wet_h = np.ascontiguousarray(
        W[:, :E].T.reshape(ET, P, D).transpose(1, 0, 2).reshape(P, ET * D)
    )
    z_all = dec_state @ W[:, E:].T + np.asarray(W_bias, dtype=np.float32)  # [B, D]
    vrep_h = np.ascontiguousarray(
        np.broadcast_to(np.asarray(v_weight, dtype=np.float32).reshape(1, D), (P, D))
    )
    ident_h = np.eye(P, dtype=np.float32)
    onesc_h = np.ones((P, 1), dtype=np.float32)

    in_maps = []
    for c in range(CORES):
        zrep_h = np.ascontiguousarray(
            np.broadcast_to(z_all[None, c * bl : (c + 1) * bl, :], (P, bl, D))
        )
        in_maps.append(
            {
                "enc": enc_out[c * bl : (c + 1) * bl],
                "wet": wet_h,
                "zrepp": zrep_h,
                "vrep": vrep_h,
                "ident": ident_h,
                "onesc": onesc_h,
            }
        )
    return in_maps


_NC_CACHE = {}


def _get_nc():
    if "nc" not in _NC_CACHE:
        _NC_CACHE["nc"] = _build_kernel()
    return _NC_CACHE["nc"]


def _run(inputs, trace=False, tmpdir=None):
    nc = _get_nc()
    in_maps = _prep_inputs(
        inputs["enc_out"],
        inputs["dec_state"],
        inputs["W_weight"],
        inputs["W_bias"],
        inputs["v_weight"],
    )
    res = run_bass_kernel_spmd(
        nc, in_maps, list(range(CORES)), trace=trace, tmpdir=tmpdir
    )
    out = np.concatenate(
        [np.asarray(res.results[c]["ctx_out"]) for c in range(CORES)], axis=0
    )
    return out.astype(np.float32, copy=False), res


def kernel(**inputs):
    out, _ = _run(inputs, trace=False)
    return out



# revision 2
# speedup vs baseline: 1.4388x; 1.4388x over previous
"""Trainium2 Bass kernel for additive (Bahdanau) attention.

  context[b] = sum_t softmax_t( v . tanh(We @ enc[b,t] + Wd @ dec[b] + bias) ) * enc[b,t]

Shapes (hardcoded): enc_out [64, 2048, 1024] f32, dec_state [64, 1024] f32,
W_weight [1024, 2048], W_bias [1024], v_weight [1, 1024].  Output [64, 1024].

Sharding: data-parallel over batch across 8 NeuronCores (8 batches/core).

Design (v2, fp8 DoubleRow):
- Host prep: enc is cast twice — bf16 in [b, tl, i, e] layout (ctx matmul
  stream) and fp8-e4m3 in pre-transposed [b, el, i, j, tl] layout (proj
  stationary), so no on-device transposes at all.  We^T is cast to fp8 in
  [el, j, d] pair layout; z = Wd@dec + bias is split z8 + zr8 (fp8 residual
  pair) and replicated across 128 partitions.
- proj = X @ We^T runs as fp8 MatmulPerfMode.DoubleRow (2 K-tiles per
  instruction, 0.5 cycles/row = 4x over the f32r baseline).  The z bias is
  accumulated into PSUM first via a DoubleRow matmul against (1/128)*ones,
  so no DVE add is needed.
- Epilogue per 128-row tile: ACT tanh reads PSUM f32 -> SBUF bf16; DVE
  scalar_tensor_tensor (x v, 2x mode on bf16) reduces to scores; ACT exp
  emits bf16 softmax weights; PE accumulates ctx += p^T @ X in bf16.
  Softmax needs no max-subtraction (|scores| <= sum|v| <= 32).
- One global software pipeline over 128 row-tiles; DMA batched 4 tiles per
  instruction (all runs >= 1KB contiguous).
"""

import sys

sys.path.insert(0, "/opt/trn_rl_repo")

from contextlib import ExitStack

import ml_dtypes
import numpy as np

import concourse.bass as bass
import concourse.tile as tile
from concourse import bacc, mybir
from concourse.bass_utils import run_bass_kernel_spmd

F32 = mybir.dt.float32
BF16 = mybir.dt.bfloat16
FP8 = mybir.dt.float8e4
NP_FP8 = ml_dtypes.float8_e4m3
NP_BF16 = ml_dtypes.bfloat16
DR = mybir.MatmulPerfMode.DoubleRow

B, T, E, D = 64, 2048, 1024, 1024
CORES = 8
BL = B // CORES           # batches per core (8)
P = 128                   # partitions
TT = T // P               # t-tiles per batch (16)
ET = E // P               # e-blocks per row-tile (8)
QUAD = 4                  # t-tiles fetched per DMA instruction
CTX_LAG = 2               # t-tiles of lag before emitting ctx matmuls
PREFETCH_QUADS = 3


def _build_kernel(bl=BL, t_tiles=TT):
    nc = bacc.Bacc(
        "TRN2",
        target_bir_lowering=False,
        debug=False,
        num_devices=CORES,
    )

    # [b, tl, i, e]: x16[b, tl, i, :] = enc[b, i*128+tl, :] in bf16
    x16 = nc.declare_dram_parameter("x16", [bl, P, t_tiles, E], BF16, isOutput=False)
    # [b, el, i, j, tl]: xt8[b, el, i, j, tl] = enc[b, i*128+tl, j*128+el] in fp8
    xt8 = nc.declare_dram_parameter("xt8", [bl, P, t_tiles, ET, P], FP8, isOutput=False)
    # [el, j, d]: wet8[el, j, d] = We[d, j*128+el] in fp8
    wet8 = nc.declare_dram_parameter("wet8", [P, ET, D], FP8, isOutput=False)
    # [k, b, 2, d]: (z8, zr8) replicated over k partitions
    zpair8 = nc.declare_dram_parameter("zpair8", [P, bl, 2, D], FP8, isOutput=False)
    ones8 = nc.declare_dram_parameter("ones8", [P, 2, P], FP8, isOutput=False)
    v16 = nc.declare_dram_parameter("v16", [P, D], BF16, isOutput=False)
    onesc = nc.declare_dram_parameter("onesc", [P, 1], F32, isOutput=False)
    out = nc.declare_dram_parameter("ctx_out", [bl, E], F32, isOutput=True)

    n_quads_total = bl * t_tiles // QUAD

    with tile.TileContext(nc) as tc, ExitStack() as ctx:
        const = ctx.enter_context(tc.tile_pool(name="const", bufs=1))
        xq_pool = ctx.enter_context(tc.tile_pool(name="xq", bufs=3))
        xtq_pool = ctx.enter_context(tc.tile_pool(name="xtq", bufs=3))
        epool = ctx.enter_context(tc.tile_pool(name="e", bufs=3))
        small = ctx.enter_context(tc.tile_pool(name="small", bufs=2))

        ps_proj = ctx.enter_context(tc.tile_pool(name="ps_proj", bufs=2, space="PSUM"))
        ps_ctx = ctx.enter_context(tc.tile_pool(name="ps_ctx", bufs=2, space="PSUM"))
        ps_misc = ctx.enter_context(tc.tile_pool(name="ps_misc", bufs=2, space="PSUM"))

        # ---- resident constants.  Ordered so step-0 dependencies land first.
        ones_sb = const.tile([P, 2, P], FP8)
        nc.sync.dma_start(ones_sb[:], ones8[:])
        z_sb = const.tile([P, bl, 2, D], FP8)
        nc.sync.dma_start(z_sb[:, 0], zpair8[:, 0])
        wet_sb = const.tile([P, ET, D], FP8)
        nc.sync.dma_start(wet_sb[:], wet8[:])

        xq_tiles = {}
        xtq_tiles = {}

        def fetch_quad(q):
            b, qi = divmod(q, t_tiles // QUAD)
            xq = xq_pool.tile([P, QUAD, E], BF16, tag="xq")
            nc.sync.dma_start(xq[:], x16[b, :, QUAD * qi : QUAD * (qi + 1), :])
            xq_tiles[q] = xq
            xtq = xtq_pool.tile([P, QUAD, ET, P], FP8, tag="xtq")
            nc.sync.dma_start(xtq[:], xt8[b, :, QUAD * qi : QUAD * (qi + 1), :, :])
            xtq_tiles[q] = xtq

        fetch_quad(0)
        v_sb = const.tile([P, D], BF16)
        nc.sync.dma_start(v_sb[:], v16[:])
        onesc_sb = const.tile([P, 1], F32)
        nc.sync.dma_start(onesc_sb[:], onesc[:])
        for q in range(1, PREFETCH_QUADS):
            fetch_quad(q)
        for b in range(1, bl):
            nc.sync.dma_start(z_sb[:, b], zpair8[:, b])

        # ---- per-batch state ------------------------------------------------
        total = bl * t_tiles
        state = {}

        def get_state(b):
            if b not in state:
                state[b] = dict(
                    s_all=small.tile([P, t_tiles], F32, tag="s", name=f"s_all_{b}"),
                    p_all=small.tile([P, t_tiles], BF16, tag="p", name=f"p_all_{b}"),
                    ctx0=ps_ctx.tile([1, 512], F32, tag="ps_ctx", name=f"ctx0_{b}"),
                    ctx1=ps_ctx.tile([1, 512], F32, tag="ps_ctx", name=f"ctx1_{b}"),
                    proj_ps=[None] * t_tiles,
                )
            return state[b]

        def emit_proj(b, i):
            # proj[t, d] = z[b, d] + sum_e x[t, e] WeT[e, d], fp8 DoubleRow
            st = get_state(b)
            k = b * t_tiles + i
            q, qi = divmod(k, QUAD)
            xtq = xtq_tiles[q]
            proj = ps_proj.tile([P, D], F32, tag="ps_proj")
            st["proj_ps"][i] = proj
            for h in range(2):
                sl = slice(h * 512, (h + 1) * 512)
                nc.tensor.matmul(
                    proj[:, sl], ones_sb[:], z_sb[:, b, :, sl],
                    start=True, stop=False, perf_mode=DR,
                )
            for pr in range(ET // 2):
                lhs = xtq[:, qi, 2 * pr : 2 * pr + 2, :]
                for h in range(2):
                    sl = slice(h * 512, (h + 1) * 512)
                    nc.tensor.matmul(
                        proj[:, sl], lhs, wet_sb[:, 2 * pr : 2 * pr + 2, sl],
                        start=False, stop=(pr == ET // 2 - 1), perf_mode=DR,
                    )

        def emit_epilogue(b, i):
            # energy = tanh(proj); s = sum_d energy*v; p = exp(s)
            st = get_state(b)
            proj = st["proj_ps"][i]
            st["proj_ps"][i] = None
            e_sb = epool.tile([P, D], BF16, tag="e")
            nc.scalar.activation(e_sb[:], proj[:], mybir.ActivationFunctionType.Tanh)
            nc.vector.scalar_tensor_tensor(
                out=e_sb[:],
                in0=e_sb[:],
                scalar=1.0,
                in1=v_sb[:],
                op0=mybir.AluOpType.mult,
                op1=mybir.AluOpType.mult,
                accum_out=st["s_all"][:, i : i + 1],
            )
            nc.scalar.activation(
                st["p_all"][:, i : i + 1],
                st["s_all"][:, i : i + 1],
                mybir.ActivationFunctionType.Exp,
            )

        def emit_ctx(b, i):
            # ctx_unnorm += p^T @ X  (contraction over the 128 t-rows), bf16
            st = get_state(b)
            k = b * t_tiles + i
            q, qi = divmod(k, QUAD)
            xq = xq_tiles[q]
            p_col = st["p_all"][:, i : i + 1]
            nc.tensor.matmul(
                st["ctx0"][:], p_col, xq[:, qi, 0:512],
                start=(i == 0), stop=(i == t_tiles - 1),
            )
            nc.tensor.matmul(
                st["ctx1"][:], p_col, xq[:, qi, 512:E],
                start=(i == 0), stop=(i == t_tiles - 1),
            )
            if i == t_tiles - 1:
                emit_batch_end(b)

        def emit_batch_end(b):
            # l = sum_t exp(s_t); ctx = ctx_unnorm / l
            st = state.pop(b)
            l_part = small.tile([P, 1], F32, tag="lp")
            nc.vector.tensor_reduce(
                l_part[:], st["p_all"][:],
                axis=mybir.AxisListType.X, op=mybir.AluOpType.add,
            )
            l_ps = ps_misc.tile([1, 1], F32, tag="ps_misc")
            nc.tensor.matmul(l_ps[:], l_part[:], onesc_sb[:])
            linv = small.tile([1, 1], F32, tag="linv")
            nc.vector.reciprocal(linv[:], l_ps[:])
            ctx_row = small.tile([1, E], F32, tag="ctxrow")
            nc.scalar.activation(
                ctx_row[:, 0:512], st["ctx0"][:],
                mybir.ActivationFunctionType.Copy, scale=linv[:],
            )
            nc.scalar.activation(
                ctx_row[:, 512:E], st["ctx1"][:],
                mybir.ActivationFunctionType.Copy, scale=linv[:],
            )
            nc.sync.dma_start(out[b : b + 1, :], ctx_row[:])

        # ---- main software pipeline over all (batch, t-tile) ----------------
        for k in range(total + CTX_LAG):
            if k % QUAD == 0:
                qf = k // QUAD + PREFETCH_QUADS
                if qf < n_quads_total:
                    fetch_quad(qf)
            if k < total:
                emit_proj(*divmod(k, t_tiles))
            if 0 <= k - CTX_LAG:
                emit_ctx(*divmod(k - CTX_LAG, t_tiles))
            if 0 <= k - 1 < total:
                emit_epilogue(*divmod(k - 1, t_tiles))

    nc.compile()
    return nc


def _prep_inputs(enc_out, dec_state, W_weight, W_bias, v_weight, bl=BL):
    """Host-side layout/dtype prep + per-core slicing."""
    enc_out = np.ascontiguousarray(enc_out, dtype=np.float32)
    dec_state = np.ascontiguousarray(dec_state, dtype=np.float32)
    W = np.asarray(W_weight, dtype=np.float32)

    # x16: [B, tl, i, e] bf16
    x16_h = np.ascontiguousarray(
        enc_out.reshape(B, TT, P, E).transpose(0, 2, 1, 3).astype(NP_BF16)
    )
    # xt8: [B, el, i, j, tl] fp8
    enc8 = enc_out.astype(NP_FP8)
    xt8_h = np.ascontiguousarray(
        enc8.reshape(B, TT, P, ET, P).transpose(0, 4, 1, 3, 2)
    )
    # wet8: [el, j, d]
    wet8_h = np.ascontiguousarray(
        W[:, :E].T.astype(NP_FP8).reshape(ET, P, D).transpose(1, 0, 2)
    )
    # z = Wd @ dec + bias, split into fp8 + fp8 residual, replicated over k
    z_all = dec_state @ W[:, E:].T + np.asarray(W_bias, dtype=np.float32)  # [B, D]
    z8 = z_all.astype(NP_FP8)
    zr8 = (z_all - z8.astype(np.float32)).astype(NP_FP8)
    zpair = np.stack([z8, zr8], axis=1)  # [B, 2, D]
    ones_h = np.full((P, 2, P), 1.0 / 128.0, dtype=NP_FP8)
    v16_h = np.ascontiguousarray(
        np.broadcast_to(np.asarray(v_weight).astype(NP_BF16).reshape(1, D), (P, D))
    )
    onesc_h = np.ones((P, 1), dtype=np.float32)

    in_maps = []
    for c in range(CORES):
        sl = slice(c * bl, (c + 1) * bl)
        zpair_h = np.ascontiguousarray(
            np.broadcast_to(zpair[None, sl], (P, bl, 2, D))
        )
        in_maps.append(
            {
                "x16": x16_h[sl],
                "xt8": xt8_h[sl],
                "wet8": wet8_h,
                "zpair8": zpair_h,
                "ones8": ones_h,
                "v16": v16_h,
                "onesc": onesc_h,
            }
        )
    return in_maps


_NC_CACHE = {}


def _get_nc():
    if "nc" not in _NC_CACHE:
        _NC_CACHE["nc"] = _build_kernel()
    return _NC_CACHE["nc"]


def _run(inputs, trace=False, tmpdir=None):
    nc = _get_nc()
    in_maps = _prep_inputs(
        inputs["enc_out"],
        inputs["dec_state"],
        inputs["W_weight"],
        inputs["W_bias"],
        inputs["v_weight"],
    )
    res = run_bass_kernel_spmd(
        nc, in_maps, list(range(CORES)), trace=trace, tmpdir=tmpdir
    )
    out = np.concatenate(
        [np.asarray(res.results[c]["ctx_out"]) for c in range(CORES)], axis=0
    )
    return out.astype(np.float32, copy=False), res


def kernel(**inputs):
    out, _ = _run(inputs, trace=False)
    return out


if __name__ == "__main__":
    pass


# revision 12
# speedup vs baseline: 1.8734x; 1.3021x over previous
"""Trainium2 Bass kernel for additive (Bahdanau) attention.

  context[b] = sum_t softmax_t( v . tanh(We @ enc[b,t] + Wd @ dec[b] + bias) ) * enc[b,t]

Shapes (hardcoded): enc_out [64, 2048, 1024] f32, dec_state [64, 1024] f32,
W_weight [1024, 2048], W_bias [1024], v_weight [1, 1024].  Output [64, 1024].

Sharding: data-parallel over batch across 8 NeuronCores (8 batches/core).

Design (v2, fp8 DoubleRow):
- Host prep: enc is cast twice — bf16 in [b, tl, i, e] layout (ctx matmul
  stream) and fp8-e4m3 in pre-transposed [b, el, i, j, tl] layout (proj
  stationary), so no on-device transposes at all.  We^T is cast to fp8 in
  [el, j, d] pair layout; z = Wd@dec + bias is split z8 + zr8 (fp8 residual
  pair) and replicated across 128 partitions.
- proj = X @ (32*We^T) runs as fp8 MatmulPerfMode.DoubleRow (2 K-tiles per
  instruction; on HW each N=512 matmul costs ~259ns regardless of K, so DR
  halves the pass count: 8 matmuls/tile).  The x32 weight scaling keeps the
  small We values out of e4m3's subnormal range (quantization error there
  otherwise dominates) and is undone exactly by tanh's scale=1/32.
- z bias (scaled x32, bf16) is added to the PSUM output on DVE (Pool/GPSIMD
  cannot access PSUM), keeping the z-add off the bottleneck PE.
- Epilogue per 128-row tile: DVE add (PSUM f32 + zrep -> SBUF bf16); ACT
  tanh(scale=1/32); Pool scalar_tensor_tensor (x v) reduces to scores; ACT
  exp emits bf16 softmax weights; PE accumulates ctx += p^T @ X in bf16.
  Softmax needs no max-subtraction (|scores| <= sum|v| <= 32).
- One global software pipeline over 128 row-tiles; DMA batched 4 tiles per
  instruction (all runs >= 1KB contiguous).
"""

import sys

sys.path.insert(0, "/opt/trn_rl_repo")

from contextlib import ExitStack

import ml_dtypes
import numpy as np

import concourse.bass as bass
import concourse.tile as tile
from concourse import bacc, mybir
from concourse.bass_utils import run_bass_kernel_spmd

F32 = mybir.dt.float32
BF16 = mybir.dt.bfloat16
FP8 = mybir.dt.float8e4
NP_FP8 = ml_dtypes.float8_e4m3
NP_BF16 = ml_dtypes.bfloat16
DR = mybir.MatmulPerfMode.DoubleRow

B, T, E, D = 64, 2048, 1024, 1024
CORES = 8
BL = B // CORES           # batches per core (8)
P = 128                   # partitions
TT = T // P               # t-tiles per batch (16)
ET = E // P               # e-blocks per row-tile (8)
QUAD = 4                  # t-tiles fetched per DMA instruction
CTX_LAG = 3               # t-tiles of lag before emitting ctx matmuls
PREFETCH_QUADS = 3
WSCALE = 32.0             # fp8 subnormal-avoidance scale on We^T and z


def _build_kernel(bl=BL, t_tiles=TT):
    nc = bacc.Bacc(
        "TRN2",
        target_bir_lowering=False,
        debug=False,
        num_devices=CORES,
    )

    # [b, tl, i, e]: x16[b, tl, i, :] = enc[b, i*128+tl, :] in bf16
    x16 = nc.declare_dram_parameter("x16", [bl, P, t_tiles, E], BF16, isOutput=False)
    # [b, el, i, j, tl]: xt8[b, el, i, j, tl] = enc[b, i*128+tl, j*128+el] in fp8
    xt8 = nc.declare_dram_parameter("xt8", [bl, P, t_tiles, ET, P], FP8, isOutput=False)
    # [el, j, d]: wet8[el, j, d] = 32 * We[d, j*128+el] in fp8
    wet8 = nc.declare_dram_parameter("wet8", [P, ET, D], FP8, isOutput=False)
    # [k, b, d]: 32 * z[b, d] in bf16, replicated over k partitions
    zrep16 = nc.declare_dram_parameter("zrep16", [P, bl, D], BF16, isOutput=False)
    v16 = nc.declare_dram_parameter("v16", [P, D], BF16, isOutput=False)
    onesc = nc.declare_dram_parameter("onesc", [P, 1], F32, isOutput=False)
    out = nc.declare_dram_parameter("ctx_out", [bl, E], F32, isOutput=True)

    n_quads_total = bl * t_tiles // QUAD

    with tile.TileContext(nc) as tc, ExitStack() as ctx:
        const = ctx.enter_context(tc.tile_pool(name="const", bufs=1))
        xq_pool = ctx.enter_context(tc.tile_pool(name="xq", bufs=3))
        xtq_pool = ctx.enter_context(tc.tile_pool(name="xtq", bufs=3))
        epool = ctx.enter_context(tc.tile_pool(name="e", bufs=3))
        small = ctx.enter_context(tc.tile_pool(name="small", bufs=2))

        ps_proj = ctx.enter_context(tc.tile_pool(name="ps_proj", bufs=2, space="PSUM"))
        ps_ctx = ctx.enter_context(tc.tile_pool(name="ps_ctx", bufs=2, space="PSUM"))
        ps_misc = ctx.enter_context(tc.tile_pool(name="ps_misc", bufs=2, space="PSUM"))

        # ---- resident constants.  Ordered so step-0 dependencies land first.
        z_sb = const.tile([P, bl, D], BF16)
        nc.sync.dma_start(z_sb[:, 0], zrep16[:, 0])
        wet_sb = const.tile([P, ET, D], FP8)
        nc.sync.dma_start(wet_sb[:], wet8[:])

        xq_tiles = {}
        xtq_tiles = {}

        def fetch_quad(q):
            b, qi = divmod(q, t_tiles // QUAD)
            xq = xq_pool.tile([P, QUAD, E], BF16, tag="xq")
            nc.sync.dma_start(xq[:], x16[b, :, QUAD * qi : QUAD * (qi + 1), :])
            xq_tiles[q] = xq
            xtq = xtq_pool.tile([P, QUAD, ET, P], FP8, tag="xtq")
            nc.sync.dma_start(xtq[:], xt8[b, :, QUAD * qi : QUAD * (qi + 1), :, :])
            xtq_tiles[q] = xtq

        fetch_quad(0)
        v_sb = const.tile([P, D], BF16)
        nc.sync.dma_start(v_sb[:], v16[:])
        onesc_sb = const.tile([P, 1], F32)
        nc.sync.dma_start(onesc_sb[:], onesc[:])
        for q in range(1, PREFETCH_QUADS):
            fetch_quad(q)
        for b in range(1, bl):
            nc.sync.dma_start(z_sb[:, b], zrep16[:, b])

        # ---- per-batch state ------------------------------------------------
        total = bl * t_tiles
        state = {}

        def get_state(b):
            if b not in state:
                state[b] = dict(
                    s_all=small.tile([P, t_tiles], F32, tag="s", name=f"s_all_{b}"),
                    p_all=small.tile([P, t_tiles], BF16, tag="p", name=f"p_all_{b}"),
                    ctx0=ps_ctx.tile([1, 512], F32, tag="ps_ctx", name=f"ctx0_{b}"),
                    ctx1=ps_ctx.tile([1, 512], F32, tag="ps_ctx", name=f"ctx1_{b}"),
                    proj_ps=[None] * t_tiles,
                )
            return state[b]

        def emit_proj(b, i):
            # proj[t, d] = sum_e x[t, e] * 32*WeT[e, d], fp8 DoubleRow
            st = get_state(b)
            k = b * t_tiles + i
            q, qi = divmod(k, QUAD)
            xtq = xtq_tiles[q]
            proj = ps_proj.tile([P, D], F32, tag="ps_proj")
            st["proj_ps"][i] = proj
            for pr in range(ET // 2):
                lhs = xtq[:, qi, 2 * pr : 2 * pr + 2, :]
                for h in range(2):
                    sl = slice(h * 512, (h + 1) * 512)
                    nc.tensor.matmul(
                        proj[:, sl], lhs, wet_sb[:, 2 * pr : 2 * pr + 2, sl],
                        start=(pr == 0), stop=(pr == ET // 2 - 1), perf_mode=DR,
                    )

        def emit_epilogue(b, i):
            # energy = tanh((proj + 32z)/32); s = sum_d energy*v; p = exp(s)
            st = get_state(b)
            proj = st["proj_ps"][i]
            st["proj_ps"][i] = None
            e_sb = epool.tile([P, D], BF16, tag="e")
            nc.vector.tensor_add(e_sb[:], proj[:], z_sb[:, b, :])
            nc.scalar.activation(
                e_sb[:], e_sb[:], mybir.ActivationFunctionType.Tanh,
                scale=1.0 / WSCALE,
            )
            nc.vector.scalar_tensor_tensor(
                out=e_sb[:],
                in0=e_sb[:],
                scalar=1.0,
                in1=v_sb[:],
                op0=mybir.AluOpType.mult,
                op1=mybir.AluOpType.mult,
                accum_out=st["s_all"][:, i : i + 1],
            )
            nc.scalar.activation(
                st["p_all"][:, i : i + 1],
                st["s_all"][:, i : i + 1],
                mybir.ActivationFunctionType.Exp,
            )

        def emit_ctx(b, i):
            # ctx_unnorm += p^T @ X  (contraction over the 128 t-rows), bf16
            st = get_state(b)
            k = b * t_tiles + i
            q, qi = divmod(k, QUAD)
            xq = xq_tiles[q]
            p_col = st["p_all"][:, i : i + 1]
            nc.tensor.matmul(
                st["ctx0"][:], p_col, xq[:, qi, 0:512],
                start=(i == 0), stop=(i == t_tiles - 1),
            )
            nc.tensor.matmul(
                st["ctx1"][:], p_col, xq[:, qi, 512:E],
                start=(i == 0), stop=(i == t_tiles - 1),
            )
            if i == t_tiles - 1:
                emit_batch_end(b)

        def emit_batch_end(b):
            # l = sum_t exp(s_t); ctx = ctx_unnorm / l
            st = state.pop(b)
            l_part = small.tile([P, 1], F32, tag="lp")
            nc.vector.tensor_reduce(
                l_part[:], st["p_all"][:],
                axis=mybir.AxisListType.X, op=mybir.AluOpType.add,
            )
            l_ps = ps_misc.tile([1, 1], F32, tag="ps_misc")
            nc.tensor.matmul(l_ps[:], l_part[:], onesc_sb[:])
            linv = small.tile([1, 1], F32, tag="linv")
            nc.vector.reciprocal(linv[:], l_ps[:])
            ctx_row = small.tile([1, E], F32, tag="ctxrow")
            nc.scalar.activation(
                ctx_row[:, 0:512], st["ctx0"][:],
                mybir.ActivationFunctionType.Copy, scale=linv[:],
            )
            nc.scalar.activation(
                ctx_row[:, 512:E], st["ctx1"][:],
                mybir.ActivationFunctionType.Copy, scale=linv[:],
            )
            nc.sync.dma_start(out[b : b + 1, :], ctx_row[:])

        # ---- main software pipeline over all (batch, t-tile) ----------------
        for k in range(total + CTX_LAG):
            if k % QUAD == 0:
                qf = k // QUAD + PREFETCH_QUADS
                if qf < n_quads_total:
                    fetch_quad(qf)
            if k < total:
                emit_proj(*divmod(k, t_tiles))
            if 0 <= k - CTX_LAG:
                emit_ctx(*divmod(k - CTX_LAG, t_tiles))
            if 0 <= k - 1 < total:
                emit_epilogue(*divmod(k - 1, t_tiles))

    nc.compile()
    return nc


def _prep_inputs(enc_out, dec_state, W_weight, W_bias, v_weight, bl=BL):
    """Host-side layout/dtype prep + per-core slicing."""
    enc_out = np.ascontiguousarray(enc_out, dtype=np.float32)
    dec_state = np.ascontiguousarray(dec_state, dtype=np.float32)
    W = np.asarray(W_weight, dtype=np.float32)

    # x16: [B, tl, i, e] bf16
    x16_h = np.ascontiguousarray(
        enc_out.reshape(B, TT, P, E).transpose(0, 2, 1, 3).astype(NP_BF16)
    )
    # xt8: [B, el, i, j, tl] fp8
    enc8 = enc_out.astype(NP_FP8)
    xt8_h = np.ascontiguousarray(
        enc8.reshape(B, TT, P, ET, P).transpose(0, 4, 1, 3, 2)
    )
    # wet8: [el, j, d], scaled by WSCALE to avoid e4m3 subnormals
    wet8_h = np.ascontiguousarray(
        (WSCALE * W[:, :E].T).astype(NP_FP8).reshape(ET, P, D).transpose(1, 0, 2)
    )
    # z = Wd @ dec + bias, scaled by WSCALE, bf16, replicated over k
    z_all = dec_state @ W[:, E:].T + np.asarray(W_bias, dtype=np.float32)  # [B, D]
    z16 = (WSCALE * z_all).astype(NP_BF16)
    v16_h = np.ascontiguousarray(
        np.broadcast_to(np.asarray(v_weight).astype(NP_BF16).reshape(1, D), (P, D))
    )
    onesc_h = np.ones((P, 1), dtype=np.float32)

    in_maps = []
    for c in range(CORES):
        sl = slice(c * bl, (c + 1) * bl)
        zrep_h = np.ascontiguousarray(np.broadcast_to(z16[None, sl], (P, bl, D)))
        in_maps.append(
            {
                "x16": x16_h[sl],
                "xt8": xt8_h[sl],
                "wet8": wet8_h,
                "zrep16": zrep_h,
                "v16": v16_h,
                "onesc": onesc_h,
            }
        )
    return in_maps


_NC_CACHE = {}


def _get_nc():
    if "nc" not in _NC_CACHE:
        _NC_CACHE["nc"] = _build_kernel()
    return _NC_CACHE["nc"]


def _run(inputs, trace=False, tmpdir=None):
    nc = _get_nc()
    in_maps = _prep_inputs(
        inputs["enc_out"],
        inputs["dec_state"],
        inputs["W_weight"],
        inputs["W_bias"],
        inputs["v_weight"],
    )
    res = run_bass_kernel_spmd(
        nc, in_maps, list(range(CORES)), trace=trace, tmpdir=tmpdir
    )
    out = np.concatenate(
        [np.asarray(res.results[c]["ctx_out"]) for c in range(CORES)], axis=0
    )
    return out.astype(np.float32, copy=False), res


def kernel(**inputs):
    out, _ = _run(inputs, trace=False)
    return out


if __name__ == "__main__":
    pass


# revision 16
# speedup vs baseline: 1.8872x; 1.0074x over previous
"""Trainium2 Bass kernel for additive (Bahdanau) attention.

  context[b] = sum_t softmax_t( v . tanh(We @ enc[b,t] + Wd @ dec[b] + bias) ) * enc[b,t]

Shapes (hardcoded): enc_out [64, 2048, 1024] f32, dec_state [64, 1024] f32,
W_weight [1024, 2048], W_bias [1024], v_weight [1, 1024].  Output [64, 1024].

Sharding: data-parallel over batch across 8 NeuronCores (8 batches/core).

Design (v2, fp8 DoubleRow):
- Host prep: enc is cast twice — bf16 in [b, tl, i, e] layout (ctx matmul
  stream) and fp8-e4m3 in pre-transposed [b, el, i, j, tl] layout (proj
  stationary), so no on-device transposes at all.  We^T is cast to fp8 in
  [el, j, d] pair layout; z = Wd@dec + bias is split z8 + zr8 (fp8 residual
  pair) and replicated across 128 partitions.
- proj = X @ (32*We^T) runs as fp8 MatmulPerfMode.DoubleRow (2 K-tiles per
  instruction; on HW each N=512 matmul costs ~259ns regardless of K, so DR
  halves the pass count: 8 matmuls/tile).  The x32 weight scaling keeps the
  small We values out of e4m3's subnormal range (quantization error there
  otherwise dominates) and is undone exactly by tanh's scale=1/32.
- z bias (scaled x32, bf16) is added to the PSUM output on DVE (Pool/GPSIMD
  cannot access PSUM), keeping the z-add off the bottleneck PE.
- Epilogue per 128-row tile: DVE add (PSUM f32 + zrep -> SBUF bf16); ACT
  tanh(scale=1/32); Pool scalar_tensor_tensor (x v) reduces to scores; ACT
  exp emits bf16 softmax weights; PE accumulates ctx += p^T @ X in bf16.
  Softmax needs no max-subtraction (|scores| <= sum|v| <= 32).
- One global software pipeline over 128 row-tiles; DMA batched 4 tiles per
  instruction (all runs >= 1KB contiguous).
"""

import sys

sys.path.insert(0, "/opt/trn_rl_repo")

from contextlib import ExitStack

import ml_dtypes
import numpy as np

import concourse.bass as bass
import concourse.tile as tile
from concourse import bacc, mybir
from concourse.bass_utils import run_bass_kernel_spmd

F32 = mybir.dt.float32
BF16 = mybir.dt.bfloat16
FP8 = mybir.dt.float8e4
NP_FP8 = ml_dtypes.float8_e4m3
NP_BF16 = ml_dtypes.bfloat16
DR = mybir.MatmulPerfMode.DoubleRow

B, T, E, D = 64, 2048, 1024, 1024
CORES = 8
BL = B // CORES           # batches per core (8)
P = 128                   # partitions
TT = T // P               # t-tiles per batch (16)
ET = E // P               # e-blocks per row-tile (8)
QUAD = 4                  # t-tiles fetched per DMA instruction
CTX_LAG = 4               # t-tiles of lag before emitting ctx matmuls
EXP_LAG = 3               # t-tiles of lag before emitting exp (vs proj)
END_LAG = 2               # extra t-tiles before emitting batch-end chain
PREFETCH_QUADS = 3
WSCALE = 32.0             # fp8 subnormal-avoidance scale on We^T and z


def _build_kernel(bl=BL, t_tiles=TT):
    nc = bacc.Bacc(
        "TRN2",
        target_bir_lowering=False,
        debug=False,
        num_devices=CORES,
    )

    # [b, tl, i, e]: x16[b, tl, i, :] = enc[b, i*128+tl, :] in bf16
    x16 = nc.declare_dram_parameter("x16", [bl, P, t_tiles, E], BF16, isOutput=False)
    # [b, el, i, j, tl]: xt8[b, el, i, j, tl] = enc[b, i*128+tl, j*128+el] in fp8
    xt8 = nc.declare_dram_parameter("xt8", [bl, P, t_tiles, ET, P], FP8, isOutput=False)
    # [el, j, d]: wet8[el, j, d] = 32 * We[d, j*128+el] in fp8
    wet8 = nc.declare_dram_parameter("wet8", [P, ET, D], FP8, isOutput=False)
    # [k, b, d]: 32 * z[b, d] in bf16, replicated over k partitions
    zrep16 = nc.declare_dram_parameter("zrep16", [P, bl, D], BF16, isOutput=False)
    v16 = nc.declare_dram_parameter("v16", [P, D], BF16, isOutput=False)
    onesc = nc.declare_dram_parameter("onesc", [P, 1], F32, isOutput=False)
    out = nc.declare_dram_parameter("ctx_out", [bl, E], F32, isOutput=True)

    n_quads_total = bl * t_tiles // QUAD

    with tile.TileContext(nc) as tc, ExitStack() as ctx:
        const = ctx.enter_context(tc.tile_pool(name="const", bufs=1))
        xq_pool = ctx.enter_context(tc.tile_pool(name="xq", bufs=4))
        xtq_pool = ctx.enter_context(tc.tile_pool(name="xtq", bufs=4))
        epool = ctx.enter_context(tc.tile_pool(name="e", bufs=4))
        small = ctx.enter_context(tc.tile_pool(name="small", bufs=2))

        ps_proj = ctx.enter_context(tc.tile_pool(name="ps_proj", bufs=2, space="PSUM"))
        ps_ctx = ctx.enter_context(tc.tile_pool(name="ps_ctx", bufs=2, space="PSUM"))
        ps_misc = ctx.enter_context(tc.tile_pool(name="ps_misc", bufs=2, space="PSUM"))

        # ---- resident constants.  Ordered so step-0 dependencies land first.
        z_sb = const.tile([P, bl, D], BF16)
        nc.sync.dma_start(z_sb[:, 0], zrep16[:, 0])
        wet_sb = const.tile([P, ET, D], FP8)
        nc.sync.dma_start(wet_sb[:], wet8[:])

        xq_tiles = {}
        xtq_tiles = {}

        def fetch_quad(q):
            b, qi = divmod(q, t_tiles // QUAD)
            xq = xq_pool.tile([P, QUAD, E], BF16, tag="xq")
            nc.sync.dma_start(xq[:], x16[b, :, QUAD * qi : QUAD * (qi + 1), :])
            xq_tiles[q] = xq
            xtq = xtq_pool.tile([P, QUAD, ET, P], FP8, tag="xtq")
            nc.sync.dma_start(xtq[:], xt8[b, :, QUAD * qi : QUAD * (qi + 1), :, :])
            xtq_tiles[q] = xtq

        fetch_quad(0)
        v_sb = const.tile([P, D], BF16)
        nc.sync.dma_start(v_sb[:], v16[:])
        onesc_sb = const.tile([P, 1], F32)
        nc.sync.dma_start(onesc_sb[:], onesc[:])
        for q in range(1, PREFETCH_QUADS):
            fetch_quad(q)
        for b in range(1, bl):
            nc.sync.dma_start(z_sb[:, b], zrep16[:, b])

        # ---- per-batch state ------------------------------------------------
        total = bl * t_tiles
        state = {}

        def get_state(b):
            if b not in state:
                state[b] = dict(
                    s_all=small.tile([P, t_tiles], F32, tag="s", name=f"s_all_{b}"),
                    p_all=small.tile([P, t_tiles], BF16, tag="p", name=f"p_all_{b}"),
                    ctx0=ps_ctx.tile([1, 512], F32, tag="ps_ctx", name=f"ctx0_{b}"),
                    ctx1=ps_ctx.tile([1, 512], F32, tag="ps_ctx", name=f"ctx1_{b}"),
                    proj_ps=[None] * t_tiles,
                )
            return state[b]

        def emit_proj(b, i):
            # proj[t, d] = sum_e x[t, e] * 32*WeT[e, d], fp8 DoubleRow
            st = get_state(b)
            k = b * t_tiles + i
            q, qi = divmod(k, QUAD)
            xtq = xtq_tiles[q]
            proj = ps_proj.tile([P, D], F32, tag="ps_proj")
            st["proj_ps"][i] = proj
            for pr in range(ET // 2):
                lhs = xtq[:, qi, 2 * pr : 2 * pr + 2, :]
                for h in range(2):
                    sl = slice(h * 512, (h + 1) * 512)
                    nc.tensor.matmul(
                        proj[:, sl], lhs, wet_sb[:, 2 * pr : 2 * pr + 2, sl],
                        start=(pr == 0), stop=(pr == ET // 2 - 1), perf_mode=DR,
                    )

        def emit_epilogue(b, i):
            # energy = tanh((proj + 32z)/32); s = sum_d energy*v
            st = get_state(b)
            proj = st["proj_ps"][i]
            st["proj_ps"][i] = None
            e_sb = epool.tile([P, D], BF16, tag="e")
            nc.vector.tensor_add(e_sb[:], proj[:], z_sb[:, b, :])
            nc.scalar.activation(
                e_sb[:], e_sb[:], mybir.ActivationFunctionType.Tanh,
                scale=1.0 / WSCALE,
            )
            nc.vector.scalar_tensor_tensor(
                out=e_sb[:],
                in0=e_sb[:],
                scalar=1.0,
                in1=v_sb[:],
                op0=mybir.AluOpType.mult,
                op1=mybir.AluOpType.mult,
                accum_out=st["s_all"][:, i : i + 1],
            )

        def emit_exp(b, i):
            # p = exp(s); deferred so this ACT instr never blocks a tanh
            st = get_state(b)
            nc.scalar.activation(
                st["p_all"][:, i : i + 1],
                st["s_all"][:, i : i + 1],
                mybir.ActivationFunctionType.Exp,
            )

        def emit_ctx(b, i):
            # ctx_unnorm += p^T @ X  (contraction over the 128 t-rows), bf16
            st = get_state(b)
            k = b * t_tiles + i
            q, qi = divmod(k, QUAD)
            xq = xq_tiles[q]
            p_col = st["p_all"][:, i : i + 1]
            nc.tensor.matmul(
                st["ctx0"][:], p_col, xq[:, qi, 0:512],
                start=(i == 0), stop=(i == t_tiles - 1),
            )
            nc.tensor.matmul(
                st["ctx1"][:], p_col, xq[:, qi, 512:E],
                start=(i == 0), stop=(i == t_tiles - 1),
            )

        def emit_batch_end(b):
            # l = sum_t exp(s_t); ctx = ctx_unnorm / l
            st = state.pop(b)
            l_part = small.tile([P, 1], F32, tag="lp")
            nc.vector.tensor_reduce(
                l_part[:], st["p_all"][:],
                axis=mybir.AxisListType.X, op=mybir.AluOpType.add,
            )
            l_ps = ps_misc.tile([1, 1], F32, tag="ps_misc")
            nc.tensor.matmul(l_ps[:], l_part[:], onesc_sb[:])
            linv = small.tile([1, 1], F32, tag="linv")
            nc.vector.reciprocal(linv[:], l_ps[:])
            ctx_row = small.tile([1, E], F32, tag="ctxrow")
            nc.scalar.activation(
                ctx_row[:, 0:512], st["ctx0"][:],
                mybir.ActivationFunctionType.Copy, scale=linv[:],
            )
            nc.scalar.activation(
                ctx_row[:, 512:E], st["ctx1"][:],
                mybir.ActivationFunctionType.Copy, scale=linv[:],
            )
            nc.sync.dma_start(out[b : b + 1, :], ctx_row[:])

        # ---- main software pipeline over all (batch, t-tile) ----------------
        for k in range(total + CTX_LAG + END_LAG):
            if k % QUAD == 0:
                qf = k // QUAD + PREFETCH_QUADS
                if qf < n_quads_total:
                    fetch_quad(qf)
            if k < total:
                emit_proj(*divmod(k, t_tiles))
            if 0 <= k - CTX_LAG < total:
                emit_ctx(*divmod(k - CTX_LAG, t_tiles))
            if 0 <= k - 1 < total:
                emit_epilogue(*divmod(k - 1, t_tiles))
            if 0 <= k - EXP_LAG < total:
                emit_exp(*divmod(k - EXP_LAG, t_tiles))
            kb = k - CTX_LAG - END_LAG
            if 0 <= kb < total and kb % t_tiles == t_tiles - 1:
                emit_batch_end(kb // t_tiles)

    nc.compile()
    return nc


def _prep_inputs(enc_out, dec_state, W_weight, W_bias, v_weight, bl=BL):
    """Host-side layout/dtype prep + per-core slicing."""
    enc_out = np.ascontiguousarray(enc_out, dtype=np.float32)
    dec_state = np.ascontiguousarray(dec_state, dtype=np.float32)
    W = np.asarray(W_weight, dtype=np.float32)

    # x16: [B, tl, i, e] bf16
    x16_h = np.ascontiguousarray(
        enc_out.reshape(B, TT, P, E).transpose(0, 2, 1, 3).astype(NP_BF16)
    )
    # xt8: [B, el, i, j, tl] fp8
    enc8 = enc_out.astype(NP_FP8)
    xt8_h = np.ascontiguousarray(
        enc8.reshape(B, TT, P, ET, P).transpose(0, 4, 1, 3, 2)
    )
    # wet8: [el, j, d], scaled by WSCALE to avoid e4m3 subnormals
    wet8_h = np.ascontiguousarray(
        (WSCALE * W[:, :E].T).astype(NP_FP8).reshape(ET, P, D).transpose(1, 0, 2)
    )
    # z = Wd @ dec + bias, scaled by WSCALE, bf16, replicated over k
    z_all = dec_state @ W[:, E:].T + np.asarray(W_bias, dtype=np.float32)  # [B, D]
    z16 = (WSCALE * z_all).astype(NP_BF16)
    v16_h = np.ascontiguousarray(
        np.broadcast_to(np.asarray(v_weight).astype(NP_BF16).reshape(1, D), (P, D))
    )
    onesc_h = np.ones((P, 1), dtype=np.float32)

    in_maps = []
    for c in range(CORES):
        sl = slice(c * bl, (c + 1) * bl)
        zrep_h = np.ascontiguousarray(np.broadcast_to(z16[None, sl], (P, bl, D)))
        in_maps.append(
            {
                "x16": x16_h[sl],
                "xt8": xt8_h[sl],
                "wet8": wet8_h,
                "zrep16": zrep_h,
                "v16": v16_h,
                "onesc": onesc_h,
            }
        )
    return in_maps


_NC_CACHE = {}


def _get_nc():
    if "nc" not in _NC_CACHE:
        _NC_CACHE["nc"] = _build_kernel()
    return _NC_CACHE["nc"]


def _run(inputs, trace=False, tmpdir=None):
    nc = _get_nc()
    in_maps = _prep_inputs(
        inputs["enc_out"],
        inputs["dec_state"],
        inputs["W_weight"],
        inputs["W_bias"],
        inputs["v_weight"],
    )
    res = run_bass_kernel_spmd(
        nc, in_maps, list(range(CORES)), trace=trace, tmpdir=tmpdir
    )
    out = np.concatenate(
        [np.asarray(res.results[c]["ctx_out"]) for c in range(CORES)], axis=0
    )
    return out.astype(np.float32, copy=False), res


def kernel(**inputs):
    out, _ = _run(inputs, trace=False)
    return out


if __name__ == "__main__":
    pass


# revision 18
# speedup vs baseline: 1.9830x; 1.0507x over previous
"""Trainium2 Bass kernel for additive (Bahdanau) attention.

  context[b] = sum_t softmax_t( v . tanh(We @ enc[b,t] + Wd @ dec[b] + bias) ) * enc[b,t]

Shapes (hardcoded): enc_out [64, 2048, 1024] f32, dec_state [64, 1024] f32,
W_weight [1024, 2048], W_bias [1024], v_weight [1, 1024].  Output [64, 1024].

Sharding: data-parallel over batch across 8 NeuronCores (8 batches/core).

Design (v2, fp8 DoubleRow):
- Host prep: enc is cast twice — bf16 in [b, tl, i, e] layout (ctx matmul
  stream) and fp8-e4m3 in pre-transposed [b, el, i, j, tl] layout (proj
  stationary), so no on-device transposes at all.  We^T is cast to fp8 in
  [el, j, d] pair layout; z = Wd@dec + bias is split z8 + zr8 (fp8 residual
  pair) and replicated across 128 partitions.
- proj = X @ (32*We^T) runs as fp8 MatmulPerfMode.DoubleRow (2 K-tiles per
  instruction; on HW each N=512 matmul costs ~259ns regardless of K, so DR
  halves the pass count: 8 matmuls/tile).  The x32 weight scaling keeps the
  small We values out of e4m3's subnormal range (quantization error there
  otherwise dominates) and is undone exactly by tanh's scale=1/32.
- z bias (scaled x32, bf16) is added to the PSUM output on DVE (Pool/GPSIMD
  cannot access PSUM), keeping the z-add off the bottleneck PE.
- Epilogue per 128-row tile: DVE add (PSUM f32 + zrep -> SBUF bf16); ACT
  tanh(scale=1/32); Pool scalar_tensor_tensor (x v) reduces to scores; ACT
  exp emits bf16 softmax weights; PE accumulates ctx += p^T @ X in bf16.
  Softmax needs no max-subtraction (|scores| <= sum|v| <= 32).
- One global software pipeline over 128 row-tiles; DMA batched 4 tiles per
  instruction (all runs >= 1KB contiguous).
"""

import sys

sys.path.insert(0, "/opt/trn_rl_repo")

from contextlib import ExitStack

import ml_dtypes
import numpy as np

import concourse.bass as bass
import concourse.tile as tile
from concourse import bacc, mybir
from concourse.bass_utils import run_bass_kernel_spmd

F32 = mybir.dt.float32
BF16 = mybir.dt.bfloat16
FP8 = mybir.dt.float8e4
NP_FP8 = ml_dtypes.float8_e4m3
NP_BF16 = ml_dtypes.bfloat16
DR = mybir.MatmulPerfMode.DoubleRow

B, T, E, D = 64, 2048, 1024, 1024
CORES = 8
BL = B // CORES           # batches per core (8)
P = 128                   # partitions
TT = T // P               # t-tiles per batch (16)
ET = E // P               # e-blocks per row-tile (8)
QUAD = 4                  # t-tiles fetched per DMA instruction
CTX_LAG = 4               # t-tiles of lag before emitting ctx matmuls
EXP_LAG = 3               # t-tiles of lag before emitting exp (vs proj)
END_LAG = 2               # extra t-tiles before emitting batch-end chain
PREFETCH_QUADS = 3
WSCALE = 32.0             # fp8 subnormal-avoidance scale on We^T and z


def _build_kernel(bl=BL, t_tiles=TT):
    nc = bacc.Bacc(
        "TRN2",
        target_bir_lowering=False,
        debug=False,
        num_devices=CORES,
    )

    # [b, tl, i, e]: x16[b, tl, i, :] = enc[b, i*128+tl, :] in bf16
    x16 = nc.declare_dram_parameter("x16", [bl, P, t_tiles, E], BF16, isOutput=False)
    # [b, el, i, j, tl]: xt8[b, el, i, j, tl] = enc[b, i*128+tl, j*128+el] in fp8
    xt8 = nc.declare_dram_parameter("xt8", [bl, P, t_tiles, ET, P], FP8, isOutput=False)
    # [el, j, d]: wet8[el, j, d] = 32 * We[d, j*128+el] in fp8
    wet8 = nc.declare_dram_parameter("wet8", [P, ET, D], FP8, isOutput=False)
    # [k, b, d]: 32 * z[b, d] in bf16, replicated over k partitions
    zrep16 = nc.declare_dram_parameter("zrep16", [P, bl, D], BF16, isOutput=False)
    v16 = nc.declare_dram_parameter("v16", [P, D], BF16, isOutput=False)
    onesc = nc.declare_dram_parameter("onesc", [P, 1], F32, isOutput=False)
    out = nc.declare_dram_parameter("ctx_out", [bl, E], F32, isOutput=True)

    n_quads_total = bl * t_tiles // QUAD

    with tile.TileContext(nc) as tc, ExitStack() as ctx:
        const = ctx.enter_context(tc.tile_pool(name="const", bufs=1))
        xq_pool = ctx.enter_context(tc.tile_pool(name="xq", bufs=4))
        xtq_pool = ctx.enter_context(tc.tile_pool(name="xtq", bufs=4))
        epool = ctx.enter_context(tc.tile_pool(name="e", bufs=4))
        small = ctx.enter_context(tc.tile_pool(name="small", bufs=2))

        ps_proj = ctx.enter_context(tc.tile_pool(name="ps_proj", bufs=2, space="PSUM"))
        ps_ctx = ctx.enter_context(tc.tile_pool(name="ps_ctx", bufs=2, space="PSUM"))
        ps_misc = ctx.enter_context(tc.tile_pool(name="ps_misc", bufs=2, space="PSUM"))

        # ---- resident constants.  Ordered so step-0 dependencies land first.
        z_sb = const.tile([P, bl, D], BF16)
        nc.sync.dma_start(z_sb[:, 0], zrep16[:, 0])
        wet_sb = const.tile([P, ET, D], FP8)
        nc.sync.dma_start(wet_sb[:], wet8[:])

        xq_tiles = {}
        xtq_tiles = {}

        def fetch_quad(q):
            b, qi = divmod(q, t_tiles // QUAD)
            xq = xq_pool.tile([P, QUAD, E], BF16, tag="xq")
            nc.sync.dma_start(xq[:], x16[b, :, QUAD * qi : QUAD * (qi + 1), :])
            xq_tiles[q] = xq
            xtq = xtq_pool.tile([P, QUAD, ET, P], FP8, tag="xtq")
            nc.sync.dma_start(xtq[:], xt8[b, :, QUAD * qi : QUAD * (qi + 1), :, :])
            xtq_tiles[q] = xtq

        fetch_quad(0)
        v_sb = const.tile([P, D], BF16)
        nc.sync.dma_start(v_sb[:], v16[:])
        onesc_sb = const.tile([P, 1], F32)
        nc.sync.dma_start(onesc_sb[:], onesc[:])
        for q in range(1, PREFETCH_QUADS):
            fetch_quad(q)
        for b in range(1, bl):
            nc.sync.dma_start(z_sb[:, b], zrep16[:, b])

        # ---- per-batch state ------------------------------------------------
        total = bl * t_tiles
        state = {}

        def get_state(b):
            if b not in state:
                state[b] = dict(
                    s_all=small.tile([P, t_tiles], F32, tag="s", name=f"s_all_{b}"),
                    s1_all=small.tile([P, t_tiles], F32, tag="s1", name=f"s1_all_{b}"),
                    p_all=small.tile([P, t_tiles], BF16, tag="p", name=f"p_all_{b}"),
                    ctx0=ps_ctx.tile([1, 512], F32, tag="ps_ctx", name=f"ctx0_{b}"),
                    ctx1=ps_ctx.tile([1, 512], F32, tag="ps_ctx", name=f"ctx1_{b}"),
                    proj_ps=[None] * t_tiles,
                )
            return state[b]

        def emit_proj(b, i):
            # proj[t, d] = sum_e x[t, e] * 32*WeT[e, d], fp8 DoubleRow
            st = get_state(b)
            k = b * t_tiles + i
            q, qi = divmod(k, QUAD)
            xtq = xtq_tiles[q]
            proj = ps_proj.tile([P, D], F32, tag="ps_proj")
            st["proj_ps"][i] = proj
            for pr in range(ET // 2):
                lhs = xtq[:, qi, 2 * pr : 2 * pr + 2, :]
                for h in range(2):
                    sl = slice(h * 512, (h + 1) * 512)
                    nc.tensor.matmul(
                        proj[:, sl], lhs, wet_sb[:, 2 * pr : 2 * pr + 2, sl],
                        start=(pr == 0), stop=(pr == ET // 2 - 1), perf_mode=DR,
                    )

        H = 640  # DVE handles cols [0:H) of the v-dot; Pool+ACT handle [H:D)

        def emit_epilogue(b, i):
            # energy = tanh((proj + 32z)/32); s = sum_d energy*v, split so the
            # v-dot load is spread over DVE (stt) and Pool*ACT (mult+reduce)
            st = get_state(b)
            proj = st["proj_ps"][i]
            st["proj_ps"][i] = None
            e_sb = epool.tile([P, D], BF16, tag="e")
            nc.vector.tensor_add(e_sb[:], proj[:], z_sb[:, b, :])
            nc.scalar.activation(
                e_sb[:], e_sb[:], mybir.ActivationFunctionType.Tanh,
                scale=1.0 / WSCALE,
            )
            nc.vector.scalar_tensor_tensor(
                out=e_sb[:, 0:H],
                in0=e_sb[:, 0:H],
                scalar=1.0,
                in1=v_sb[:, 0:H],
                op0=mybir.AluOpType.mult,
                op1=mybir.AluOpType.mult,
                accum_out=st["s_all"][:, i : i + 1],
            )
            nc.gpsimd.tensor_tensor(
                e_sb[:, H:D], e_sb[:, H:D], v_sb[:, H:D], mybir.AluOpType.mult
            )
            nc.scalar.activation(
                e_sb[:, H:D], e_sb[:, H:D], mybir.ActivationFunctionType.Copy,
                accum_out=st["s1_all"][:, i : i + 1],
            )

        def emit_exp(b, i):
            # p = exp(s0 + s1); deferred so this ACT instr never blocks a tanh
            st = get_state(b)
            nc.scalar.activation(
                st["p_all"][:, i : i + 1],
                st["s_all"][:, i : i + 1],
                mybir.ActivationFunctionType.Exp,
                bias=st["s1_all"][:, i : i + 1],
            )

        def emit_ctx(b, i):
            # ctx_unnorm += p^T @ X  (contraction over the 128 t-rows), bf16
            st = get_state(b)
            k = b * t_tiles + i
            q, qi = divmod(k, QUAD)
            xq = xq_tiles[q]
            p_col = st["p_all"][:, i : i + 1]
            nc.tensor.matmul(
                st["ctx0"][:], p_col, xq[:, qi, 0:512],
                start=(i == 0), stop=(i == t_tiles - 1),
            )
            nc.tensor.matmul(
                st["ctx1"][:], p_col, xq[:, qi, 512:E],
                start=(i == 0), stop=(i == t_tiles - 1),
            )

        def emit_batch_end(b):
            # l = sum_t exp(s_t); ctx = ctx_unnorm / l
            st = state.pop(b)
            l_part = small.tile([P, 1], F32, tag="lp")
            nc.vector.tensor_reduce(
                l_part[:], st["p_all"][:],
                axis=mybir.AxisListType.X, op=mybir.AluOpType.add,
            )
            l_ps = ps_misc.tile([1, 1], F32, tag="ps_misc")
            nc.tensor.matmul(l_ps[:], l_part[:], onesc_sb[:])
            linv = small.tile([1, 1], F32, tag="linv")
            nc.vector.reciprocal(linv[:], l_ps[:])
            ctx_row = small.tile([1, E], F32, tag="ctxrow")
            nc.scalar.activation(
                ctx_row[:, 0:512], st["ctx0"][:],
                mybir.ActivationFunctionType.Copy, scale=linv[:],
            )
            nc.scalar.activation(
                ctx_row[:, 512:E], st["ctx1"][:],
                mybir.ActivationFunctionType.Copy, scale=linv[:],
            )
            nc.sync.dma_start(out[b : b + 1, :], ctx_row[:])

        # ---- main software pipeline over all (batch, t-tile) ----------------
        for k in range(total + CTX_LAG + END_LAG):
            if k % QUAD == 0:
                qf = k // QUAD + PREFETCH_QUADS
                if qf < n_quads_total:
                    fetch_quad(qf)
            if k < total:
                emit_proj(*divmod(k, t_tiles))
            if 0 <= k - CTX_LAG < total:
                emit_ctx(*divmod(k - CTX_LAG, t_tiles))
            if 0 <= k - 1 < total:
                emit_epilogue(*divmod(k - 1, t_tiles))
            if 0 <= k - EXP_LAG < total:
                emit_exp(*divmod(k - EXP_LAG, t_tiles))
            kb = k - CTX_LAG - END_LAG
            if 0 <= kb < total and kb % t_tiles == t_tiles - 1:
                emit_batch_end(kb // t_tiles)

    nc.compile()
    return nc


def _prep_inputs(enc_out, dec_state, W_weight, W_bias, v_weight, bl=BL):
    """Host-side layout/dtype prep + per-core slicing."""
    enc_out = np.ascontiguousarray(enc_out, dtype=np.float32)
    dec_state = np.ascontiguousarray(dec_state, dtype=np.float32)
    W = np.asarray(W_weight, dtype=np.float32)

    # x16: [B, tl, i, e] bf16
    x16_h = np.ascontiguousarray(
        enc_out.reshape(B, TT, P, E).transpose(0, 2, 1, 3).astype(NP_BF16)
    )
    # xt8: [B, el, i, j, tl] fp8
    enc8 = enc_out.astype(NP_FP8)
    xt8_h = np.ascontiguousarray(
        enc8.reshape(B, TT, P, ET, P).transpose(0, 4, 1, 3, 2)
    )
    # wet8: [el, j, d], scaled by WSCALE to avoid e4m3 subnormals
    wet8_h = np.ascontiguousarray(
        (WSCALE * W[:, :E].T).astype(NP_FP8).reshape(ET, P, D).transpose(1, 0, 2)
    )
    # z = Wd @ dec + bias, scaled by WSCALE, bf16, replicated over k
    z_all = dec_state @ W[:, E:].T + np.asarray(W_bias, dtype=np.float32)  # [B, D]
    z16 = (WSCALE * z_all).astype(NP_BF16)
    v16_h = np.ascontiguousarray(
        np.broadcast_to(np.asarray(v_weight).astype(NP_BF16).reshape(1, D), (P, D))
    )
    onesc_h = np.ones((P, 1), dtype=np.float32)

    in_maps = []
    for c in range(CORES):
        sl = slice(c * bl, (c + 1) * bl)
        zrep_h = np.ascontiguousarray(np.broadcast_to(z16[None, sl], (P, bl, D)))
        in_maps.append(
            {
                "x16": x16_h[sl],
                "xt8": xt8_h[sl],
                "wet8": wet8_h,
                "zrep16": zrep_h,
                "v16": v16_h,
                "onesc": onesc_h,
            }
        )
    return in_maps


_NC_CACHE = {}


def _get_nc():
    if "nc" not in _NC_CACHE:
        _NC_CACHE["nc"] = _build_kernel()
    return _NC_CACHE["nc"]


def _run(inputs, trace=False, tmpdir=None):
    nc = _get_nc()
    in_maps = _prep_inputs(
        inputs["enc_out"],
        inputs["dec_state"],
        inputs["W_weight"],
        inputs["W_bias"],
        inputs["v_weight"],
    )
    res = run_bass_kernel_spmd(
        nc, in_maps, list(range(CORES)), trace=trace, tmpdir=tmpdir
    )
    out = np.concatenate(
        [np.asarray(res.results[c]["ctx_out"]) for c in range(CORES)], axis=0
    )
    return out.astype(np.float32, copy=False), res


def kernel(**inputs):
    out, _ = _run(inputs, trace=False)
    return out


if __name__ == "__main__":
    pass
